# revision 47
# baseline (speedup 1.0000x reference)
"""Head-parallel causal attention for nn_PlasmidLMAttention on 8 TRN2 cores.

Sharding: core c -> batch b = c//4, heads [4*(c%4), 4*(c%4)+4).
Each core computes q/k/v projections for its 512 channels, RoPE, causal
softmax attention, and a partial o_proj (row-parallel); the host sums the
4 partial outputs per batch and gathers k/v.
"""

import sys
import numpy as np

try:
    import concourse.bass as bass
    import concourse.tile as tile
    from concourse import bacc, mybir
    from concourse.bass_utils import run_bass_kernel_spmd
except ImportError:
    sys.path.insert(0, "/opt/trn_rl_repo")
    import concourse.bass as bass
    import concourse.tile as tile
    from concourse import bacc, mybir
    from concourse.bass_utils import run_bass_kernel_spmd

P = 128
B, S, D = 2, 2048, 2048
H = 16
HD = 128
HLOC = 4          # heads per core
CH = HLOC * HD    # 512 channels per core
SB = 512          # s-block width
NB = S // SB      # 4 s-blocks
NDC = D // P      # 16 contraction chunks
NSC = S // P      # 16 s chunks
F32 = mybir.dt.float32
F32R = mybir.dt.float32r
AF = mybir.ActivationFunctionType
ALU = mybir.AluOpType
USE_F32R = True
DT = F32R if USE_F32R else F32

_COMPILED = None


def _emit(nc):
    mm = nc.tensor.matmul

    xT = nc.dram_tensor("xT", [D, S], DT, kind="ExternalInput").ap()
    wqT = nc.dram_tensor("wqT", [D, CH], DT, kind="ExternalInput").ap()
    wkT = nc.dram_tensor("wkT", [D, CH], DT, kind="ExternalInput").ap()
    wvT = nc.dram_tensor("wvT", [D, CH], DT, kind="ExternalInput").ap()
    woT = nc.dram_tensor("woT", [CH, D], DT, kind="ExternalInput").ap()
    cc = nc.dram_tensor("cc", [P, S], F32, kind="ExternalInput").ap()
    ss = nc.dram_tensor("ss", [P, S], F32, kind="ExternalInput").ap()
    msk = nc.dram_tensor("msk", [P, 4 * SB], F32, kind="ExternalInput").ap()
    onesk_d = nc.dram_tensor("onesk", [P, 1], DT, kind="ExternalInput").ap()
    onesb_d = nc.dram_tensor("onesb", [1, P], DT, kind="ExternalInput").ap()
    k_out = nc.dram_tensor("k_out", [CH, S], DT, kind="ExternalOutput").ap()
    v_out = nc.dram_tensor("v_out", [S, CH], DT, kind="ExternalOutput").ap()
    o_out = nc.dram_tensor("o_out", [S, D], F32, kind="ExternalOutput").ap()

    with tile.TileContext(nc) as tc:
        with tc.tile_pool(name="const", bufs=1) as constp, \
             tc.tile_pool(name="resident", bufs=1) as resp:
            ones_k = constp.tile([P, 1], DT)
            ones_b = constp.tile([1, P], DT)

            q_sb = resp.tile([P, HLOC * S], DT)       # [hd, h*S + s]
            k_sb = resp.tile([P, HLOC * S], DT)       # [hd, h*S + s]
            v_sb = resp.tile([P, NSC * CH], DT)       # [s%128, sc*CH + ch]

            QDC = 4                       # d-chunks per xs tile
            NQ = NDC // QDC

            def wload(w_sb, w_dram, qtr):
                nc.sync.dma_start(
                    w_sb[:, qtr * QDC * CH:(qtr + 1) * QDC * CH]
                    .rearrange("p (dc ch) -> p dc ch", dc=QDC),
                    w_dram[qtr * QDC * P:(qtr + 1) * QDC * P, :]
                    .rearrange("(dc p) ch -> p dc ch", dc=QDC))

            # ---- stage 1: merged Q+K projection + rope ----
            # pool open order fixes SBUF layout: V-stage tiles (wv, xv) will
            # land on wq/wk regions whose last readers retire early, so V
            # loads overlap the QK tail instead of waiting for rope evicts
            with tc.tile_pool(name="w1", bufs=1) as wp, \
                 tc.tile_pool(name="xs", bufs=2) as xsp, \
                 tc.tile_pool(name="ev", bufs=7) as evp, \
                 tc.tile_pool(name="rope", bufs=1) as ropep, \
                 tc.tile_pool(name="psq", bufs=1, space="PSUM") as psqp, \
                 tc.tile_pool(name="psk", bufs=1, space="PSUM") as pskp:
                wq_sb = wp.tile([P, NDC * CH], DT, name="wq_sb")
                wk_sb = wp.tile([P, NDC * CH], DT, name="wk_sb")
                cc_sb = ropep.tile([P, S], F32)
                ss_sb = ropep.tile([P, S], F32)

                def rope_p1(ps, dslc, sb_i):
                    sw = evp.tile([P, SB], F32, name="sw", tag="sw")
                    nc.scalar.activation(sw[0:64, :], ps[64:128, :], AF.Copy)
                    nc.scalar.activation(sw[64:128, :], ps[0:64, :], AF.Copy)
                    nc.vector.tensor_tensor(
                        dslc, ps[:], cc_sb[:, sb_i * SB:(sb_i + 1) * SB],
                        ALU.mult)
                    return sw

                def rope_p2(sw, dslc, sb_i):
                    nc.vector.tensor_tensor(
                        sw[:], sw[:], ss_sb[:, sb_i * SB:(sb_i + 1) * SB],
                        ALU.mult)
                    nc.gpsimd.tensor_tensor(dslc, dslc, sw[:], ALU.add)

                for sb_i in range(NB):
                    pss_q = [psqp.tile([P, SB], F32, name=f"q{h}", tag=f"q{h}")
                             for h in range(HLOC)]
                    pss_k = [pskp.tile([P, SB], F32, name=f"k{h}", tag=f"k{h}")
                             for h in range(HLOC)]

                    def qmm(i, qtr, h, xs):
                        dc = qtr * QDC + i
                        mm(pss_q[h][:],
                           wq_sb[:, dc * CH + h * HD: dc * CH + (h + 1) * HD],
                           xs[:, i * SB:(i + 1) * SB],
                           start=(dc == 0), stop=(dc == NDC - 1))

                    def kmm(i, qtr, h, xs):
                        dc = qtr * QDC + i
                        mm(pss_k[h][:],
                           wk_sb[:, dc * CH + h * HD: dc * CH + (h + 1) * HD],
                           xs[:, i * SB:(i + 1) * SB],
                           start=(dc == 0), stop=(dc == NDC - 1))

                    def qsl(h):
                        return q_sb[:, h * S + sb_i * SB: h * S + (sb_i + 1) * SB]

                    def ksl(h):
                        return k_sb[:, h * S + sb_i * SB: h * S + (sb_i + 1) * SB]

                    for qtr in range(NQ):
                        xs = xsp.tile([P, QDC * SB], DT, name="xs", tag="xs")
                        if sb_i == 0 and qtr == 0:
                            # staged per-chunk loads so the first matmul can
                            # start after ~0.75MB instead of 3MB
                            for i in range(QDC):
                                nc.sync.dma_start(
                                    xs[:, i * SB:(i + 1) * SB],
                                    xT[i * P:(i + 1) * P, 0:SB])
                                nc.sync.dma_start(
                                    wq_sb[:, i * CH:(i + 1) * CH],
                                    wqT[i * P:(i + 1) * P, :])
                                nc.sync.dma_start(
                                    wk_sb[:, i * CH:(i + 1) * CH],
                                    wkT[i * P:(i + 1) * P, :])
                        else:
                            nc.sync.dma_start(
                                xs[:].rearrange("p (i c) -> p i c", i=QDC),
                                xT[qtr * QDC * P:(qtr + 1) * QDC * P,
                                   sb_i * SB:(sb_i + 1) * SB]
                                .rearrange("(dc p) c -> p dc c", dc=QDC))
                            if sb_i == 0:
                                # per-dc weight loads: matmuls for chunk dc
                                # start 0.75MB sooner than per-quarter loads
                                for i in range(QDC):
                                    dc = qtr * QDC + i
                                    nc.sync.dma_start(
                                        wq_sb[:, dc * CH:(dc + 1) * CH],
                                        wqT[dc * P:(dc + 1) * P, :])
                                    nc.sync.dma_start(
                                        wk_sb[:, dc * CH:(dc + 1) * CH],
                                        wkT[dc * P:(dc + 1) * P, :])
                                if qtr == 1:
                                    sl = slice(0, SB)
                                    nc.sync.dma_start(cc_sb[:, sl], cc[:, sl])
                                    nc.sync.dma_start(ss_sb[:, sl], ss[:, sl])
                            elif qtr == 0:
                                sl = slice(sb_i * SB, (sb_i + 1) * SB)
                                nc.sync.dma_start(cc_sb[:, sl], cc[:, sl])
                                nc.sync.dma_start(ss_sb[:, sl], ss[:, sl])
                        if qtr < NQ - 1:
                            for i in range(QDC):
                                for h in range(HLOC):
                                    qmm(i, qtr, h, xs)
                                for h in range(HLOC):
                                    kmm(i, qtr, h, xs)
                        else:
                            for i in range(QDC):
                                for h in range(HLOC):
                                    qmm(i, qtr, h, xs)
                            sw_q = [rope_p1(pss_q[h], qsl(h), sb_i)
                                    for h in range(HLOC)]
                            for i in range(QDC):
                                for h in range(HLOC):
                                    kmm(i, qtr, h, xs)
                            sw_k = [rope_p1(pss_k[h], ksl(h), sb_i)
                                    for h in range(3)]
                            rope_p2(sw_q[0], qsl(0), sb_i)
                            sw_k.append(rope_p1(pss_k[3], ksl(3), sb_i))
                            for h in range(1, HLOC):
                                rope_p2(sw_q[h], qsl(h), sb_i)
                            for h in range(HLOC):
                                rope_p2(sw_k[h], ksl(h), sb_i)

            # ---- stage 1c: V projection ([s, ch] layout) ----
            with tc.tile_pool(name="wv", bufs=1) as wvp, \
                 tc.tile_pool(name="xv", bufs=6) as xvp, \
                 tc.tile_pool(name="psv", bufs=6, space="PSUM") as psvp:
                wv_sb = wvp.tile([P, NDC * CH], DT, name="wv_sb")

                def xv_load(sc):
                    xv = xvp.tile([P, NDC * P], DT, name="xv", tag="xv")
                    if sc == 0:
                        # interleaved fine-grain primes: first V matmul can
                        # start after 0.5MB instead of 2MB
                        for i in range(QDC):
                            nc.sync.dma_start(
                                wv_sb[:, i * CH:(i + 1) * CH],
                                wvT[i * P:(i + 1) * P, :])
                            nc.sync.dma_start(
                                xv[:, i * QDC * P:(i + 1) * QDC * P]
                                .rearrange("p (i2 c) -> p i2 c", i2=QDC),
                                xT[i * QDC * P:(i + 1) * QDC * P, 0:P]
                                .rearrange("(dc p) c -> p dc c", dc=QDC))
                    else:
                        nc.sync.dma_start(
                            xv[:].rearrange("p (i c) -> p i c", i=NDC),
                            xT[:, sc * P:(sc + 1) * P]
                            .rearrange("(dc p) c -> p dc c", dc=NDC))
                    return xv

                # stream order matches consumption: sc0-3 can make partial
                # progress while wv quarters 1-3 are still in flight
                pre = {0: xv_load(0), 1: xv_load(1)}
                wload(wv_sb, wvT, 1)
                pre[2] = xv_load(2)
                wload(wv_sb, wvT, 2)
                pre[3] = xv_load(3)
                wload(wv_sb, wvT, 3)
                for sc in range(NSC):
                    xv = pre.pop(sc) if sc in pre else xv_load(sc)
                    ps = psvp.tile([P, CH], F32, name="psv", tag="psv")
                    for dc in range(NDC):
                        mm(ps[:], xv[:, dc * P:(dc + 1) * P],
                           wv_sb[:, dc * CH:(dc + 1) * CH],
                           start=(dc == 0), stop=(dc == NDC - 1))
                    nc.scalar.activation(v_sb[:, sc * CH:(sc + 1) * CH],
                                         ps[:], AF.Copy)

            # ---- stage 2 + 3: attention, o_proj interleaved per q-block ----
            with tc.tile_pool(name="s2", bufs=1) as s2p, \
                 tc.tile_pool(name="pt", bufs=6) as ptp, \
                 tc.tile_pool(name="sm", bufs=2) as smp, \
                 tc.tile_pool(name="ot", bufs=2) as otp, \
                 tc.tile_pool(name="pss", bufs=2, space="PSUM") as pssp, \
                 tc.tile_pool(name="psa", bufs=2, space="PSUM") as psap, \
                 tc.tile_pool(name="psd", bufs=1, space="PSUM") as psdp, \
                 tc.tile_pool(name="psb", bufs=1, space="PSUM") as psbp, \
                 tc.tile_pool(name="pso", bufs=2, space="PSUM") as psop:
                attn_sb = s2p.tile([P, HLOC * S], DT)    # [hd, h*S + q]
                wo_sb = s2p.tile([P, HLOC * D], DT, name="wo_sb")
                msk_sb = s2p.tile([P, 4 * SB], F32)
                nc.sync.dma_start(ones_k[:], onesk_d[:, :])
                nc.sync.dma_start(ones_b[:], onesb_d[:, :])
                nc.sync.dma_start(msk_sb[:], msk[:, :])
                for ci in range(HLOC):
                    nc.sync.dma_start(wo_sb[:, ci * D:(ci + 1) * D],
                                      woT[ci * P:(ci + 1) * P, :])

                def stage3(jj):
                    for sc in range(4 * jj, 4 * (jj + 1)):
                        ot = otp.tile([P, D], F32, name="ot", tag="ot")
                        last = sc == NSC - 1
                        for nb in range(NB):
                            pso = psop.tile([P, SB], F32, name="pso", tag="o")
                            for ci in range(HLOC):
                                mm(pso[:],
                                   attn_sb[:, ci * S + sc * P: ci * S + (sc + 1) * P],
                                   wo_sb[:, ci * D + nb * SB: ci * D + (nb + 1) * SB],
                                   start=(ci == 0), stop=(ci == HLOC - 1))
                            nc.vector.tensor_scalar_add(
                                ot[:, nb * SB:(nb + 1) * SB], pso[:], 0.0)
                            if last:
                                nc.sync.dma_start(
                                    o_out[sc * P:(sc + 1) * P,
                                          nb * SB:(nb + 1) * SB],
                                    ot[:, nb * SB:(nb + 1) * SB])
                        if not last:
                            nc.sync.dma_start(
                                o_out[sc * P:(sc + 1) * P, :], ot[:])

                def tail(out_ps, rec, j, h):
                    bps = psbp.tile([P, SB], F32, name="bps", tag="b")
                    mm(bps[:], ones_b[:], rec[:], start=True, stop=True,
                       skip_group_check=True)
                    bsb = smp.tile([P, SB], F32, name="bsb", tag="bsb")
                    nc.vector.tensor_scalar_add(bsb[:], bps[:], 0.0)
                    nc.vector.tensor_tensor(
                        attn_sb[:, h * S + j * SB: h * S + (j + 1) * SB],
                        out_ps[:], bsb[:], ALU.mult)

                pend = None
                for j in range(NB):
                    for h in range(HLOC):
                        nkc = 4 * (j + 1)
                        out_ps = psap.tile([P, SB], F32, name="avps", tag="av")
                        den_ps = psdp.tile([1, SB], F32, name="denps", tag="den")
                        qslc = q_sb[:, h * S + j * SB: h * S + (j + 1) * SB]
                        for kc in range(nkc):
                            sps = pssp.tile([P, SB], F32, name="sps", tag="s")
                            mm(sps[:],
                               k_sb[:, h * S + kc * P: h * S + (kc + 1) * P],
                               qslc, start=True, stop=True,
                               skip_group_check=True)
                            pt = ptp.tile([P, SB], DT, name="pt", tag="pt")
                            nc.scalar.activation(pt[:], sps[:], AF.Exp)
                            if kc >= 4 * j:
                                r = kc - 4 * j
                                nc.vector.tensor_tensor(
                                    pt[:], pt[:],
                                    msk_sb[:, r * SB:(r + 1) * SB], ALU.mult)
                            mm(out_ps[:],
                               v_sb[:, kc * CH + h * HD: kc * CH + (h + 1) * HD],
                               pt[:], start=(kc == 0), stop=(kc == nkc - 1),
                               skip_group_check=True)
                            mm(den_ps[:], ones_k[:], pt[:],
                               start=(kc == 0), stop=(kc == nkc - 1),
                               skip_group_check=True)
                        rec = smp.tile([1, SB], DT, name="rec", tag="rec")
                        with nc.allow_low_precision(reason="float32r is fp32-width"):
                            nc.vector.reciprocal(rec[:], den_ps[:])
                        if pend is not None:
                            tail(*pend)
                        pend = (out_ps, rec, j, h)
                        if h == 0 and j > 0:
                            stage3(j - 1)
                            if j == 1:
                                vi = nc.sync.dma_start(
                                    v_out[:, :].rearrange(
                                        "(sc p) ch -> p sc ch", sc=NSC),
                                    v_sb[:].rearrange(
                                        "p (sc ch) -> p sc ch", sc=NSC))
                                ki = nc.sync.dma_start(
                                    k_out[:, :].rearrange(
                                        "(h hd) s -> hd h s", h=HLOC),
                                    k_sb[:].rearrange(
                                        "p (h s) -> p h s", h=HLOC))
                                # pin k_out behind v_out so the scheduler
                                # can't hoist it into the V phase (it would
                                # starve the xv input stream there)
                                ki.ins.add_dependency(
                                    vi.ins.name,
                                    mybir.DependencyInfo.SYNC_ONLY)
                tail(*pend)
                stage3(NB - 1)
    return nc


def _compile():
    global _COMPILED
    if _COMPILED is None:
        nc = bacc.Bacc("TRN2", target_bir_lowering=False, debug=False)
        _emit(nc)
        nc.compile()
        _COMPILED = nc
    return _COMPILED


# even-channel-first permutation within each head (makes interleaved rope
# contiguous rotate-half); scores are invariant, k output un-permuted on host
_PERM = np.concatenate([np.arange(0, HD, 2), np.arange(1, HD, 2)])

# diagonal-block causal masks: for relative chunk r, keep where f >= 128*r + p
_MSK = np.concatenate(
    [(np.arange(SB)[None, :] >= r * P + np.arange(P)[:, None]).astype(np.float32)
     for r in range(4)], axis=1)


def kernel(hidden_states, rope_cos, rope_sin, wq, wk, wv, wo, trace=False):
    hidden_states = np.asarray(hidden_states, np.float32)
    rope_cos = np.asarray(rope_cos, np.float32)
    rope_sin = np.asarray(rope_sin, np.float32)
    wq = np.asarray(wq, np.float32)
    wk = np.asarray(wk, np.float32)
    wv = np.asarray(wv, np.float32)
    wo = np.asarray(wo, np.float32)

    nc = _compile()

    cc_host = np.ascontiguousarray(
        np.concatenate([rope_cos.T, rope_cos.T], 0))          # [128, S]
    ss_host = np.ascontiguousarray(
        np.concatenate([-rope_sin.T, rope_sin.T], 0))         # [128, S]
    wq_s = wq / np.sqrt(np.float32(HD))

    xT_b = [np.ascontiguousarray(hidden_states[b].T) for b in range(B)]

    in_maps = []
    for c in range(8):
        b, hg = c // 4, c % 4
        rows = np.concatenate(
            [hg * CH + hl * HD + _PERM for hl in range(HLOC)])
        cols = np.arange(hg * CH, (hg + 1) * CH)
        in_maps.append({
            "xT": xT_b[b],
            "wqT": np.ascontiguousarray(wq_s[rows, :].T),
            "wkT": np.ascontiguousarray(wk[rows, :].T),
            "wvT": np.ascontiguousarray(wv[cols, :].T),
            "woT": np.ascontiguousarray(wo[:, cols].T),
            "cc": cc_host,
            "ss": ss_host,
            "msk": _MSK,
            "onesk": np.ones((P, 1), np.float32),
            "onesb": np.ones((1, P), np.float32),
        })

    res = run_bass_kernel_spmd(nc, in_maps, list(range(8)), trace=trace)

    out = np.zeros((B, S, D), np.float32)
    k_full = np.empty((B, H, S, HD), np.float32)
    v_full = np.empty((B, H, S, HD), np.float32)
    for c in range(8):
        b, hg = c // 4, c % 4
        r = res.results[c]
        out[b] += r["o_out"]
        for hl in range(HLOC):
            hgl = hg * HLOC + hl
            k_full[b, hgl][:, _PERM] = r["k_out"][hl * HD:(hl + 1) * HD, :].T
            v_full[b, hgl] = r["v_out"][:, hl * HD:(hl + 1) * HD]
    if trace:
        return (out, k_full, v_full), res
    return out, k_full, v_full


# revision 55
# speedup vs baseline: 1.0084x; 1.0084x over previous
"""Head-parallel causal attention for nn_PlasmidLMAttention on 8 TRN2 cores.

Sharding: core c -> batch b = c//4, heads [4*(c%4), 4*(c%4)+4).
Each core computes q/k/v projections for its 512 channels, RoPE, causal
softmax attention, and a partial o_proj (row-parallel); the host sums the
4 partial outputs per batch and gathers k/v.
"""

import sys
import numpy as np

try:
    import concourse.bass as bass
    import concourse.tile as tile
    from concourse import bacc, mybir
    from concourse.bass_utils import run_bass_kernel_spmd
except ImportError:
    sys.path.insert(0, "/opt/trn_rl_repo")
    import concourse.bass as bass
    import concourse.tile as tile
    from concourse import bacc, mybir
    from concourse.bass_utils import run_bass_kernel_spmd

P = 128
B, S, D = 2, 2048, 2048
H = 16
HD = 128
HLOC = 4          # heads per core
CH = HLOC * HD    # 512 channels per core
SB = 512          # s-block width
NB = S // SB      # 4 s-blocks
NDC = D // P      # 16 contraction chunks
NSC = S // P      # 16 s chunks
F32 = mybir.dt.float32
F32R = mybir.dt.float32r
AF = mybir.ActivationFunctionType
ALU = mybir.AluOpType
USE_F32R = True
DT = F32R if USE_F32R else F32

_COMPILED = None


def _emit(nc):
    mm = nc.tensor.matmul

    xT = nc.dram_tensor("xT", [D, S], DT, kind="ExternalInput").ap()
    wqT = nc.dram_tensor("wqT", [D, CH], DT, kind="ExternalInput").ap()
    wkT = nc.dram_tensor("wkT", [D, CH], DT, kind="ExternalInput").ap()
    wvT = nc.dram_tensor("wvT", [D, CH], DT, kind="ExternalInput").ap()
    woT = nc.dram_tensor("woT", [CH, D], DT, kind="ExternalInput").ap()
    cc = nc.dram_tensor("cc", [P, S], F32, kind="ExternalInput").ap()
    ss = nc.dram_tensor("ss", [P, S], F32, kind="ExternalInput").ap()
    msk = nc.dram_tensor("msk", [P, 4 * SB], F32, kind="ExternalInput").ap()
    onesk_d = nc.dram_tensor("onesk", [P, 1], DT, kind="ExternalInput").ap()
    onesb_d = nc.dram_tensor("onesb", [1, P], DT, kind="ExternalInput").ap()
    k_out = nc.dram_tensor("k_out", [CH, S], DT, kind="ExternalOutput").ap()
    v_out = nc.dram_tensor("v_out", [S, CH], DT, kind="ExternalOutput").ap()
    o_out = nc.dram_tensor("o_out", [S, D], F32, kind="ExternalOutput").ap()

    with tile.TileContext(nc) as tc:
        with tc.tile_pool(name="const", bufs=1) as constp, \
             tc.tile_pool(name="resident", bufs=1) as resp:
            ones_k = constp.tile([P, 1], DT)
            ones_b = constp.tile([1, P], DT)

            q_sb = resp.tile([P, HLOC * S], DT)       # [hd, h*S + s]
            k_sb = resp.tile([P, HLOC * S], DT)       # [hd, h*S + s]
            v_sb = resp.tile([P, NSC * CH], DT)       # [s%128, sc*CH + ch]

            QDC = 4                       # d-chunks per xs tile
            NQ = NDC // QDC

            def wload(w_sb, w_dram, qtr):
                nc.sync.dma_start(
                    w_sb[:, qtr * QDC * CH:(qtr + 1) * QDC * CH]
                    .rearrange("p (dc ch) -> p dc ch", dc=QDC),
                    w_dram[qtr * QDC * P:(qtr + 1) * QDC * P, :]
                    .rearrange("(dc p) ch -> p dc ch", dc=QDC))

            # ---- stage 1: merged Q+K projection + rope ----
            # pool open order fixes SBUF layout: V-stage tiles (wv, xv) will
            # land on wq/wk regions whose last readers retire early, so V
            # loads overlap the QK tail instead of waiting for rope evicts
            with tc.tile_pool(name="w1", bufs=1) as wp, \
                 tc.tile_pool(name="xs", bufs=2) as xsp, \
                 tc.tile_pool(name="ev", bufs=7) as evp, \
                 tc.tile_pool(name="rope", bufs=1) as ropep, \
                 tc.tile_pool(name="psq", bufs=1, space="PSUM") as psqp, \
                 tc.tile_pool(name="psk", bufs=1, space="PSUM") as pskp:
                wq_sb = wp.tile([P, NDC * CH], DT, name="wq_sb")
                wk_sb = wp.tile([P, NDC * CH], DT, name="wk_sb")
                cc_sb = ropep.tile([P, S], F32)
                ss_sb = ropep.tile([P, S], F32)

                def rope_p1(ps, dslc, sb_i):
                    sw = evp.tile([P, SB], F32, name="sw", tag="sw")
                    nc.scalar.activation(sw[0:64, :], ps[64:128, :], AF.Copy)
                    nc.scalar.activation(sw[64:128, :], ps[0:64, :], AF.Copy)
                    nc.vector.tensor_tensor(
                        dslc, ps[:], cc_sb[:, sb_i * SB:(sb_i + 1) * SB],
                        ALU.mult)
                    return sw

                def rope_p2(sw, dslc, sb_i):
                    nc.vector.tensor_tensor(
                        sw[:], sw[:], ss_sb[:, sb_i * SB:(sb_i + 1) * SB],
                        ALU.mult)
                    nc.gpsimd.tensor_tensor(dslc, dslc, sw[:], ALU.add)

                for sb_i in range(NB):
                    pss_q = [psqp.tile([P, SB], F32, name=f"q{h}", tag=f"q{h}")
                             for h in range(HLOC)]
                    pss_k = [pskp.tile([P, SB], F32, name=f"k{h}", tag=f"k{h}")
                             for h in range(HLOC)]

                    def qmm(i, qtr, h, xs):
                        dc = qtr * QDC + i
                        mm(pss_q[h][:],
                           wq_sb[:, dc * CH + h * HD: dc * CH + (h + 1) * HD],
                           xs[:, i * SB:(i + 1) * SB],
                           start=(dc == 0), stop=(dc == NDC - 1))

                    def kmm(i, qtr, h, xs):
                        dc = qtr * QDC + i
                        mm(pss_k[h][:],
                           wk_sb[:, dc * CH + h * HD: dc * CH + (h + 1) * HD],
                           xs[:, i * SB:(i + 1) * SB],
                           start=(dc == 0), stop=(dc == NDC - 1))

                    def qsl(h):
                        return q_sb[:, h * S + sb_i * SB: h * S + (sb_i + 1) * SB]

                    def ksl(h):
                        return k_sb[:, h * S + sb_i * SB: h * S + (sb_i + 1) * SB]

                    for qtr in range(NQ):
                        xs = xsp.tile([P, QDC * SB], DT, name="xs", tag="xs")
                        if sb_i == 0 and qtr == 0:
                            # staged per-chunk loads so the first matmul can
                            # start after ~0.75MB instead of 3MB
                            for i in range(QDC):
                                nc.sync.dma_start(
                                    xs[:, i * SB:(i + 1) * SB],
                                    xT[i * P:(i + 1) * P, 0:SB])
                                nc.sync.dma_start(
                                    wq_sb[:, i * CH:(i + 1) * CH],
                                    wqT[i * P:(i + 1) * P, :])
                                nc.sync.dma_start(
                                    wk_sb[:, i * CH:(i + 1) * CH],
                                    wkT[i * P:(i + 1) * P, :])
                        else:
                            nc.sync.dma_start(
                                xs[:].rearrange("p (i c) -> p i c", i=QDC),
                                xT[qtr * QDC * P:(qtr + 1) * QDC * P,
                                   sb_i * SB:(sb_i + 1) * SB]
                                .rearrange("(dc p) c -> p dc c", dc=QDC))
                            if sb_i == 0:
                                # per-dc weight loads: matmuls for chunk dc
                                # start 0.75MB sooner than per-quarter loads
                                for i in range(QDC):
                                    dc = qtr * QDC + i
                                    nc.sync.dma_start(
                                        wq_sb[:, dc * CH:(dc + 1) * CH],
                                        wqT[dc * P:(dc + 1) * P, :])
                                    nc.sync.dma_start(
                                        wk_sb[:, dc * CH:(dc + 1) * CH],
                                        wkT[dc * P:(dc + 1) * P, :])
                                if qtr == 1:
                                    sl = slice(0, SB)
                                    nc.sync.dma_start(cc_sb[:, sl], cc[:, sl])
                                    nc.sync.dma_start(ss_sb[:, sl], ss[:, sl])
                            elif qtr == 0:
                                sl = slice(sb_i * SB, (sb_i + 1) * SB)
                                nc.sync.dma_start(cc_sb[:, sl], cc[:, sl])
                                nc.sync.dma_start(ss_sb[:, sl], ss[:, sl])
                        if qtr < NQ - 1:
                            for i in range(QDC):
                                for h in range(HLOC):
                                    qmm(i, qtr, h, xs)
                                for h in range(HLOC):
                                    kmm(i, qtr, h, xs)
                        else:
                            for i in range(QDC):
                                for h in range(HLOC):
                                    qmm(i, qtr, h, xs)
                            sw_q = [rope_p1(pss_q[h], qsl(h), sb_i)
                                    for h in range(HLOC)]
                            for i in range(QDC):
                                for h in range(HLOC):
                                    kmm(i, qtr, h, xs)
                            sw_k = [rope_p1(pss_k[h], ksl(h), sb_i)
                                    for h in range(3)]
                            rope_p2(sw_q[0], qsl(0), sb_i)
                            sw_k.append(rope_p1(pss_k[3], ksl(3), sb_i))
                            for h in range(1, HLOC):
                                rope_p2(sw_q[h], qsl(h), sb_i)
                            for h in range(HLOC):
                                rope_p2(sw_k[h], ksl(h), sb_i)

            # ---- stage 1c: V projection ([s, ch] layout) ----
            with tc.tile_pool(name="wv", bufs=1) as wvp, \
                 tc.tile_pool(name="xv", bufs=6) as xvp, \
                 tc.tile_pool(name="psv", bufs=4, space="PSUM") as psvp:
                wv_sb = wvp.tile([P, NDC * CH], DT, name="wv_sb")

                def xv_load(sc):
                    xv = xvp.tile([P, NDC * P], DT, name="xv", tag="xv")
                    if sc == 0:
                        # interleaved fine-grain primes: first V matmul can
                        # start after 0.5MB instead of 2MB
                        for i in range(QDC):
                            nc.sync.dma_start(
                                wv_sb[:, i * CH:(i + 1) * CH],
                                wvT[i * P:(i + 1) * P, :])
                            nc.sync.dma_start(
                                xv[:, i * QDC * P:(i + 1) * QDC * P]
                                .rearrange("p (i2 c) -> p i2 c", i2=QDC),
                                xT[i * QDC * P:(i + 1) * QDC * P, 0:P]
                                .rearrange("(dc p) c -> p dc c", dc=QDC))
                    else:
                        nc.sync.dma_start(
                            xv[:].rearrange("p (i c) -> p i c", i=NDC),
                            xT[:, sc * P:(sc + 1) * P]
                            .rearrange("(dc p) c -> p dc c", dc=NDC))
                    return xv

                # stream order matches consumption: sc0-3 can make partial
                # progress while wv quarters 1-3 are still in flight
                pre = {0: xv_load(0), 1: xv_load(1)}
                wload(wv_sb, wvT, 1)
                pre[2] = xv_load(2)
                wload(wv_sb, wvT, 2)
                pre[3] = xv_load(3)
                wload(wv_sb, wvT, 3)
                for sc in range(NSC):
                    xv = pre.pop(sc) if sc in pre else xv_load(sc)
                    ps = psvp.tile([P, CH], F32, name="psv", tag="psv")
                    for dc in range(NDC):
                        mm(ps[:], xv[:, dc * P:(dc + 1) * P],
                           wv_sb[:, dc * CH:(dc + 1) * CH],
                           start=(dc == 0), stop=(dc == NDC - 1))
                    nc.scalar.activation(v_sb[:, sc * CH:(sc + 1) * CH],
                                         ps[:], AF.Copy)

            # ---- stage 2 + 3: attention, o_proj interleaved per q-block ----
            with tc.tile_pool(name="s2", bufs=1) as s2p, \
                 tc.tile_pool(name="pt", bufs=6) as ptp, \
                 tc.tile_pool(name="sm", bufs=2) as smp, \
                 tc.tile_pool(name="ot", bufs=2) as otp, \
                 tc.tile_pool(name="pss", bufs=2, space="PSUM") as pssp, \
                 tc.tile_pool(name="psa", bufs=2, space="PSUM") as psap, \
                 tc.tile_pool(name="psd", bufs=1, space="PSUM") as psdp, \
                 tc.tile_pool(name="psb", bufs=1, space="PSUM") as psbp, \
                 tc.tile_pool(name="pso", bufs=2, space="PSUM") as psop:
                attn_sb = s2p.tile([P, HLOC * S], DT)    # [hd, h*S + q]
                wo_sb = s2p.tile([P, HLOC * D], DT, name="wo_sb")
                msk_sb = s2p.tile([P, 4 * SB], F32)
                nc.sync.dma_start(ones_k[:], onesk_d[:, :])
                nc.sync.dma_start(ones_b[:], onesb_d[:, :])
                nc.sync.dma_start(msk_sb[:], msk[:, :])
                for ci in range(HLOC):
                    nc.sync.dma_start(wo_sb[:, ci * D:(ci + 1) * D],
                                      woT[ci * P:(ci + 1) * P, :])

                def stage3(jj):
                    for sc in range(4 * jj, 4 * (jj + 1)):
                        ot = otp.tile([P, D], F32, name="ot", tag="ot")
                        last = jj == NB - 1
                        for nb in range(NB):
                            pso = psop.tile([P, SB], F32, name="pso", tag="o")
                            for ci in range(HLOC):
                                mm(pso[:],
                                   attn_sb[:, ci * S + sc * P: ci * S + (sc + 1) * P],
                                   wo_sb[:, ci * D + nb * SB: ci * D + (nb + 1) * SB],
                                   start=(ci == 0), stop=(ci == HLOC - 1))
                            nc.vector.tensor_scalar_add(
                                ot[:, nb * SB:(nb + 1) * SB], pso[:], 0.0)
                            if last:
                                nc.sync.dma_start(
                                    o_out[sc * P:(sc + 1) * P,
                                          nb * SB:(nb + 1) * SB],
                                    ot[:, nb * SB:(nb + 1) * SB])
                        if not last:
                            nc.sync.dma_start(
                                o_out[sc * P:(sc + 1) * P, :], ot[:])

                def tail(out_ps, rec, j, h):
                    bps = psbp.tile([P, SB], F32, name="bps", tag="b")
                    mm(bps[:], ones_b[:], rec[:], start=True, stop=True,
                       skip_group_check=True)
                    bsb = smp.tile([P, SB], F32, name="bsb", tag="bsb")
                    nc.vector.tensor_scalar_add(bsb[:], bps[:], 0.0)
                    nc.vector.tensor_tensor(
                        attn_sb[:, h * S + j * SB: h * S + (j + 1) * SB],
                        out_ps[:], bsb[:], ALU.mult)

                pend = None
                for j in range(NB):
                    for h in range(HLOC):
                        nkc = 4 * (j + 1)
                        out_ps = psap.tile([P, SB], F32, name="avps", tag="av")
                        den_ps = psdp.tile([1, SB], F32, name="denps", tag="den")
                        qslc = q_sb[:, h * S + j * SB: h * S + (j + 1) * SB]
                        for kc in range(nkc):
                            sps = pssp.tile([P, SB], F32, name="sps", tag="s")
                            mm(sps[:],
                               k_sb[:, h * S + kc * P: h * S + (kc + 1) * P],
                               qslc, start=True, stop=True,
                               skip_group_check=True)
                            pt = ptp.tile([P, SB], DT, name="pt", tag="pt")
                            nc.scalar.activation(pt[:], sps[:], AF.Exp)
                            if kc >= 4 * j:
                                r = kc - 4 * j
                                nc.vector.tensor_tensor(
                                    pt[:], pt[:],
                                    msk_sb[:, r * SB:(r + 1) * SB], ALU.mult)
                            mm(out_ps[:],
                               v_sb[:, kc * CH + h * HD: kc * CH + (h + 1) * HD],
                               pt[:], start=(kc == 0), stop=(kc == nkc - 1),
                               skip_group_check=True)
                            mm(den_ps[:], ones_k[:], pt[:],
                               start=(kc == 0), stop=(kc == nkc - 1),
                               skip_group_check=True)
                        rec = smp.tile([1, SB], DT, name="rec", tag="rec")
                        with nc.allow_low_precision(reason="float32r is fp32-width"):
                            nc.vector.reciprocal(rec[:], den_ps[:])
                        if pend is not None:
                            tail(*pend)
                        pend = (out_ps, rec, j, h)
                        if h == 0 and j > 0:
                            stage3(j - 1)
                            if j == 1:
                                vi = nc.sync.dma_start(
                                    v_out[:, :].rearrange(
                                        "(sc p) ch -> p sc ch", sc=NSC),
                                    v_sb[:].rearrange(
                                        "p (sc ch) -> p sc ch", sc=NSC))
                                ki = nc.sync.dma_start(
                                    k_out[:, :].rearrange(
                                        "(h hd) s -> hd h s", h=HLOC),
                                    k_sb[:].rearrange(
                                        "p (h s) -> p h s", h=HLOC))
                                # pin k_out behind v_out so the scheduler
                                # can't hoist it into the V phase (it would
                                # starve the xv input stream there)
                                ki.ins.add_dependency(
                                    vi.ins.name,
                                    mybir.DependencyInfo.SYNC_ONLY)
                tail(*pend)
                stage3(NB - 1)
    return nc


def _compile():
    global _COMPILED
    if _COMPILED is None:
        nc = bacc.Bacc("TRN2", target_bir_lowering=False, debug=False)
        _emit(nc)
        nc.compile()
        _COMPILED = nc
    return _COMPILED


# even-channel-first permutation within each head (makes interleaved rope
# contiguous rotate-half); scores are invariant, k output un-permuted on host
_PERM = np.concatenate([np.arange(0, HD, 2), np.arange(1, HD, 2)])

# diagonal-block causal masks: for relative chunk r, keep where f >= 128*r + p
_MSK = np.concatenate(
    [(np.arange(SB)[None, :] >= r * P + np.arange(P)[:, None]).astype(np.float32)
     for r in range(4)], axis=1)


def kernel(hidden_states, rope_cos, rope_sin, wq, wk, wv, wo, trace=False):
    hidden_states = np.asarray(hidden_states, np.float32)
    rope_cos = np.asarray(rope_cos, np.float32)
    rope_sin = np.asarray(rope_sin, np.float32)
    wq = np.asarray(wq, np.float32)
    wk = np.asarray(wk, np.float32)
    wv = np.asarray(wv, np.float32)
    wo = np.asarray(wo, np.float32)

    nc = _compile()

    cc_host = np.ascontiguousarray(
        np.concatenate([rope_cos.T, rope_cos.T], 0))          # [128, S]
    ss_host = np.ascontiguousarray(
        np.concatenate([-rope_sin.T, rope_sin.T], 0))         # [128, S]
    wq_s = wq / np.sqrt(np.float32(HD))

    xT_b = [np.ascontiguousarray(hidden_states[b].T) for b in range(B)]

    in_maps = []
    for c in range(8):
        b, hg = c // 4, c % 4
        rows = np.concatenate(
            [hg * CH + hl * HD + _PERM for hl in range(HLOC)])
        cols = np.arange(hg * CH, (hg + 1) * CH)
        in_maps.append({
            "xT": xT_b[b],
            "wqT": np.ascontiguousarray(wq_s[rows, :].T),
            "wkT": np.ascontiguousarray(wk[rows, :].T),
            "wvT": np.ascontiguousarray(wv[cols, :].T),
            "woT": np.ascontiguousarray(wo[:, cols].T),
            "cc": cc_host,
            "ss": ss_host,
            "msk": _MSK,
            "onesk": np.ones((P, 1), np.float32),
            "onesb": np.ones((1, P), np.float32),
        })

    res = run_bass_kernel_spmd(nc, in_maps, list(range(8)), trace=trace)

    out = np.zeros((B, S, D), np.float32)
    k_full = np.empty((B, H, S, HD), np.float32)
    v_full = np.empty((B, H, S, HD), np.float32)
    for c in range(8):
        b, hg = c // 4, c % 4
        r = res.results[c]
        out[b] += r["o_out"]
        for hl in range(HLOC):
            hgl = hg * HLOC + hl
            k_full[b, hgl][:, _PERM] = r["k_out"][hl * HD:(hl + 1) * HD, :].T
            v_full[b, hgl] = r["v_out"][:, hl * HD:(hl + 1) * HD]
    if trace:
        return (out, k_full, v_full), res
    return out, k_full, v_full
